# revision 21
# baseline (speedup 1.0000x reference)
"""Trainium2 Bass kernel for nn_NeuralSurface (8-layer MLP SDF with harmonic
embedding + skip concat), data-parallel over 8 NeuronCores.

v2 layout: host-precomputed fp16 harmonic embedding packed two point-sets per
512-col tile (set A at partitions 0:39, set B at 64:103) so the K=39 layers
(L0 and L4's embedding chunk) run as two concurrent row-group matmuls
(row strips 0-1 / 2-3). PSUM grouped by M-half ([A|B] per 2-bank tile) so
each layer drains through one fused [128,1024] ReLU per engine with a single
per-partition bias column. Weights fp16, stationary; per-layer batched weight
DMAs ordered so compute starts ~9us into the NEFF instead of ~30.
"""

import numpy as np

import concourse.bacc as bacc
import concourse.mybir as mybir
import concourse.tile as tile
from concourse.bass_utils import run_bass_kernel_spmd

AF = mybir.ActivationFunctionType
ALU = mybir.AluOpType
F32 = mybir.dt.float32
F16 = mybir.dt.float16

N_CORES = 8
N = 262144
NPC = N // N_CORES   # 32768 points per core
NT = 512             # free-dim per matmul (PSUM bank limit, fp32)
PT = 2 * NT          # points per ptile (A set + B set)
NTILES = NPC // PT   # 32
H = 256
E = 39
NHARM = 6

_CACHED = {}


def _ts(i, size):
    return slice(i * size, (i + 1) * size)


def _build():
    nc = bacc.Bacc("TRN2")

    embp = nc.dram_tensor("embp", [128, NPC // 2], F16, kind="ExternalInput").ap()
    w0d = nc.dram_tensor("w0d", [128, 2 * 128], F16, kind="ExternalInput").ap()
    wkh = {
        i: nc.dram_tensor(f"wk{i}", [128, 2 * 256], F16, kind="ExternalInput").ap()
        for i in (1, 2, 3, 5, 6, 7)
    }
    # w4 pack: cols 0:256 dup'd E-rows chunk, 256:512 w4a, 512:768 w4b,
    # 768:770 wsdf halves
    w4p = nc.dram_tensor("w4p", [128, 770], F16, kind="ExternalInput").ap()
    # cols 0:16 = per (layer, m-half) ReLU bias; col 16 = bsdf
    biasm = nc.dram_tensor("biasm", [128, 17], F32, kind="ExternalInput").ap()
    out_o = nc.dram_tensor("out_o", [NPC // NT, NT], F32, kind="ExternalOutput").ap()

    with tile.TileContext(nc) as tc:
        with (
            tc.tile_pool(name="wp", bufs=1) as wp,
            tc.tile_pool(name="ep", bufs=4) as ep,
            tc.tile_pool(name="hp", bufs=5) as hp,
            tc.tile_pool(name="op", bufs=4) as op_,
            tc.tile_pool(name="pp", bufs=7, space="PSUM") as pp,
            tc.tile_pool(name="pf", bufs=1, space="PSUM") as pf,
        ):
            # ---- weight/const loads: embedding tile 0 first on the sync
            # queue; weights issue in parallel on the idle GpSimd queue ----
            POOL = mybir.EngineType.Pool
            embt0 = ep.tile([128, NT], F16, tag="embt")  # ptile 0 embedding
            nc.sync.dma_start(out=embt0, in_=embp[:, 0:NT])
            w0s = wp.tile_from(w0d, name="w0s", forced_dma_engine=POOL)
            bs_ = wp.tile_from(biasm, name="bs_", forced_dma_engine=POOL)
            wks = {}
            for i in (1, 2, 3):
                wks[i] = wp.tile_from(wkh[i], name=f"wks{i}",
                                      forced_dma_engine=POOL)  # [128, 512]
            w4s = wp.tile_from(w4p, name="w4s", forced_dma_engine=POOL)
            for i in (5, 6, 7):
                wks[i] = wp.tile_from(wkh[i], name=f"wks{i}",
                                      forced_dma_engine=POOL)

            def wchunk(i, c, m):
                # layer i (1..7, not 4), K-chunk c, M-half m -> [128, 128]
                return wks[i][:, _ts(2 * c + m, 128)]

            w4e_a = w4s[0:64, :]     # rows 0:39 used (A set)
            w4e_b = w4s[64:128, :]   # rows 64:103 used (B set)
            wsdf_a = w4s[:, 768:769]
            wsdf_b = w4s[:, 769:770]

            def relu_q(dst, src, li, m):
                # ReLU+bias for one [128, 512] quarter; m0 -> ACT, m1 -> DVE
                bias = bs_[:, _ts(2 * li + m, 1)]
                if m == 0:
                    nc.scalar.activation(dst, src, AF.Relu, bias=bias)
                else:
                    nc.vector.tensor_scalar(
                        dst, src, bias, 0.0, op0=ALU.add, op1=ALU.max
                    )

            pending = None  # deferred SDF drain: (psf, t)

            def flush_out(pending):
                psf, tp = pending
                # single drain op covers both chain outputs (partitions 0, 32)
                ot = op_.tile([33, NT], F32, tag="ot", name="ot")
                nc.scalar.activation(
                    ot, psf, AF.Identity, bias=bs_[0:33, 16:17]
                )
                nc.sync.dma_start(out=out_o[2 * tp:2 * tp + 1, :], in_=ot[0:1, :])
                nc.sync.dma_start(
                    out=out_o[2 * tp + 1:2 * tp + 2, :], in_=ot[32:33, :]
                )

            # tile state: t -> {"embt": tile, "h": {li: quarter dict}}
            st = {}

            def emit_layer(t, li):
                S = st[t]
                embt = S["embt"]
                h = {
                    (s, m): hp.tile(
                        [128, NT], F16, tag=f"h{s}{m}", name=f"h{s}{m}_{li}"
                    )
                    for s in range(2) for m in range(2)
                }
                if li == 0:
                    # concurrent row-group pairs (A rows 0-63, B 64-127)
                    for m in range(2):
                        pa = pp.tile([128, NT], F32, tag="ps", name=f"pA{m}")
                        pb = pp.tile([128, NT], F32, tag="ps", name=f"pB{m}")
                        nc.tensor.matmul(
                            pa, w0s[0:64, _ts(m, 128)],
                            embt[0:64, :], start=True, stop=True,
                        )
                        nc.tensor.matmul(
                            pb, w0s[64:128, _ts(m, 128)],
                            embt[64:128, :], start=True, stop=True,
                        )
                        relu_q(h[(0, m)], pa, li, m)
                        relu_q(h[(1, m)], pb, li, m)
                elif li == 4:
                    # E-chunk first (concurrent row-group pairs), opens
                    # each bank's accumulation group
                    h3 = S["h"][3]
                    ps4 = {}
                    for m in range(2):
                        ps4[(0, m)] = pp.tile(
                            [128, NT], F32, tag="ps", name=f"p4A{m}"
                        )
                        ps4[(1, m)] = pp.tile(
                            [128, NT], F32, tag="ps", name=f"p4B{m}"
                        )
                        nc.tensor.matmul(
                            ps4[(0, m)], w4e_a[:, _ts(m, 128)],
                            embt[0:64, :], start=True, stop=False,
                            skip_group_check=True,
                        )
                        nc.tensor.matmul(
                            ps4[(1, m)], w4e_b[:, _ts(m, 128)],
                            embt[64:128, :], start=True, stop=False,
                            skip_group_check=True,
                        )
                    for s in range(2):
                        for m in range(2):
                            for c in range(2):
                                lhsT = w4s[:, _ts(2 + 2 * c + m, 128)]
                                nc.tensor.matmul(
                                    ps4[(s, m)], lhsT, h3[(s, c)],
                                    start=False, stop=(c == 1),
                                    skip_group_check=True,
                                )
                            relu_q(h[(s, m)], ps4[(s, m)], li, m)
                else:
                    hprev = S["h"][li - 1]
                    for s in range(2):
                        for m in range(2):
                            pq = pp.tile(
                                [128, NT], F32, tag="ps", name=f"p{s}{m}"
                            )
                            for c in range(2):
                                nc.tensor.matmul(
                                    pq, wchunk(li, c, m), hprev[(s, c)],
                                    start=(c == 0), stop=(c == 1),
                                )
                            relu_q(h[(s, m)], pq, li, m)
                S["h"][li] = h

            def emit_sdf(t):
                # final SDF layer (M=1): both chains SEQUENTIAL into ONE psum
                # bank — A at partition 0 (col strip 0), B at partition 32
                # (col strip 1). B's start=True clears only has_written bits;
                # A's finished data survives. One drain op reads both.
                h7 = st[t]["h"][7]
                psf = pf.tile([33, NT], F32, tag="fin")
                nc.tensor.matmul(
                    psf[0:1, :], wsdf_a, h7[(0, 0)],
                    start=True, stop=False, tile_position=(0, 0),
                    skip_group_check=True,
                )
                nc.tensor.matmul(
                    psf[0:1, :], wsdf_b, h7[(0, 1)],
                    start=False, stop=True, tile_position=(0, 0),
                    skip_group_check=True,
                )
                nc.tensor.matmul(
                    psf[32:33, :], wsdf_a, h7[(1, 0)],
                    start=True, stop=False, tile_position=(0, 32),
                    skip_group_check=True,
                )
                nc.tensor.matmul(
                    psf[32:33, :], wsdf_b, h7[(1, 1)],
                    start=False, stop=True, tile_position=(0, 32),
                    skip_group_check=True,
                )
                del st[t]
                return (psf, t)

            # ---- staggered 2-tile software pipeline ----
            # tile t: L0-L3 in slot t, L4-L7 in slot t+1, SDF in slot t+2.
            st[0] = {"embt": embt0, "h": {}}
            for s in range(NTILES + 2):
                if s + 1 < NTILES:  # prefetch embedding one slot ahead
                    et = ep.tile([128, NT], F16, tag="embt", name=f"embt{s + 1}")
                    nc.sync.dma_start(out=et, in_=embp[:, _ts(s + 1, NT)])
                    st[s + 1] = {"embt": et, "h": {}}
                a, b = s, s - 1  # tile a runs L0-L3, tile b runs L4-L7
                if a < NTILES:
                    emit_layer(a, 0)
                if pending is not None:
                    flush_out(pending)
                    pending = None
                for k in range(1, 4):
                    if 0 <= b < NTILES:
                        emit_layer(b, 3 + k)
                    if a < NTILES:
                        emit_layer(a, k)
                if 0 <= b < NTILES:
                    emit_layer(b, 7)
                    pending = emit_sdf(b)
            if pending is not None:
                flush_out(pending)
    nc.compile()
    return nc


def _prep_maps(points, ws, bs, wsdf, bsdf):
    pts = np.ascontiguousarray(points, dtype=np.float32).reshape(N, 3)
    freqs = (2.0 ** np.arange(NHARM)).astype(np.float64)

    w0d = np.zeros((128, 256), np.float16)
    w0d[0:E] = ws[0].astype(np.float16)
    w0d[64:64 + E] = ws[0].astype(np.float16)
    w4p = np.zeros((128, 770), np.float16)
    w4p[0:E, 0:256] = ws[4][0:E].astype(np.float16)
    w4p[64:64 + E, 0:256] = ws[4][0:E].astype(np.float16)
    # K-chunk c, M-half m at cols 256 + (2c+m)*128
    for c in range(2):
        chunk = ws[4][E + 128 * c:E + 128 * (c + 1)].astype(np.float16)
        for m in range(2):
            w4p[:, 256 + (2 * c + m) * 128:256 + (2 * c + m + 1) * 128] = \
                chunk[:, m * 128:(m + 1) * 128]
    w4p[:, 768:769] = wsdf[0:128].astype(np.float16)
    w4p[:, 769:770] = wsdf[128:256].astype(np.float16)

    wk = {}
    for i in (1, 2, 3, 5, 6, 7):
        wi = ws[i].astype(np.float16)
        packed = np.empty((128, 512), np.float16)
        for c in range(2):
            for m in range(2):
                packed[:, (2 * c + m) * 128:(2 * c + m + 1) * 128] = \
                    wi[128 * c:128 * (c + 1), m * 128:(m + 1) * 128]
        wk[i] = packed

    biasm = np.zeros((128, 17), np.float32)
    for i in range(8):
        biasm[:, 2 * i] = bs[i][0:128]
        biasm[:, 2 * i + 1] = bs[i][128:256]
    biasm[:, 16] = float(np.ravel(bsdf)[0])

    common = {"w0d": w0d, "w4p": w4p, "biasm": biasm}
    for i in (1, 2, 3, 5, 6, 7):
        common[f"wk{i}"] = wk[i]

    in_maps = []
    for cix in range(N_CORES):
        sl = pts[cix * NPC:(cix + 1) * NPC].astype(np.float64)  # [NPC, 3]
        e18 = (sl[:, :, None] * freqs).reshape(NPC, 18)
        emb39 = np.concatenate(
            [np.sin(e18), np.cos(e18), sl], axis=1
        ).astype(np.float16)  # [NPC, 39]
        Er = np.ascontiguousarray(emb39.T).reshape(E, NTILES, 2, NT)
        embp = np.zeros((128, NPC // 2), np.float16)
        embp[0:E] = Er[:, :, 0, :].reshape(E, -1)
        embp[64:64 + E] = Er[:, :, 1, :].reshape(E, -1)
        m = dict(common)
        m["embp"] = embp
        in_maps.append(m)
    return in_maps


def _prep_in_maps(inputs):
    ws = [np.asarray(inputs[f"w{i}"], dtype=np.float32) for i in range(8)]
    bs = [np.asarray(inputs[f"b{i}"], dtype=np.float32) for i in range(8)]
    return _prep_maps(
        np.asarray(inputs["points"]), ws, bs,
        np.asarray(inputs["wsdf"], dtype=np.float32),
        np.asarray(inputs["bsdf"], dtype=np.float32),
    )


def kernel(
    points, w0, b0, w1, b1, w2, b2, w3, b3, w4, b4, w5, b5, w6, b6, w7, b7,
    wsdf, bsdf,
):
    ws = [np.asarray(w, dtype=np.float32) for w in (w0, w1, w2, w3, w4, w5, w6, w7)]
    bs = [np.asarray(b, dtype=np.float32) for b in (b0, b1, b2, b3, b4, b5, b6, b7)]
    in_maps = _prep_maps(
        np.asarray(points), ws, bs,
        np.asarray(wsdf, dtype=np.float32), np.asarray(bsdf, dtype=np.float32),
    )

    if "nc" not in _CACHED:
        _CACHED["nc"] = _build()
    nc = _CACHED["nc"]

    res = run_bass_kernel_spmd(nc, in_maps, core_ids=list(range(N_CORES)))
    out = np.concatenate(
        [res.results[c]["out_o"] for c in range(N_CORES)], axis=0
    ).reshape(N, 1).astype(np.float32)
    return out


# revision 25
# speedup vs baseline: 1.1681x; 1.1681x over previous
"""Trainium2 Bass kernel for nn_NeuralSurface (8-layer MLP SDF with harmonic
embedding + skip concat), data-parallel over 8 NeuronCores.

v2 layout: host-precomputed fp16 harmonic embedding packed two point-sets per
512-col tile (set A at partitions 0:39, set B at 64:103) so the K=39 layers
(L0 and L4's embedding chunk) run as two concurrent row-group matmuls
(row strips 0-1 / 2-3). PSUM grouped by M-half ([A|B] per 2-bank tile) so
each layer drains through one fused [128,1024] ReLU per engine with a single
per-partition bias column. Weights fp16, stationary; per-layer batched weight
DMAs ordered so compute starts ~9us into the NEFF instead of ~30.
"""

import numpy as np

import concourse.bacc as bacc
import concourse.mybir as mybir
import concourse.tile as tile
from concourse.bass_utils import run_bass_kernel_spmd

AF = mybir.ActivationFunctionType
ALU = mybir.AluOpType
F32 = mybir.dt.float32
F16 = mybir.dt.float16

N_CORES = 8
N = 262144
NPC = N // N_CORES   # 32768 points per core
NT = 512             # free-dim per matmul (PSUM bank limit, fp32)
PT = 2 * NT          # points per ptile (A set + B set)
NTILES = NPC // PT   # 32
H = 256
E = 39
NHARM = 6

_CACHED = {}


def _ts(i, size):
    return slice(i * size, (i + 1) * size)


def _build():
    nc = bacc.Bacc("TRN2")

    embp = nc.dram_tensor("embp", [128, NPC // 2], F16, kind="ExternalInput").ap()
    w0d = nc.dram_tensor("w0d", [128, 2 * 128], F16, kind="ExternalInput").ap()
    wkh = {
        i: nc.dram_tensor(f"wk{i}", [128, 2 * 256], F16, kind="ExternalInput").ap()
        for i in (1, 2, 3, 5, 6, 7)
    }
    # w4 pack: cols 0:256 dup'd E-rows chunk, 256:512 w4a, 512:768 w4b,
    # 768:770 wsdf halves
    w4p = nc.dram_tensor("w4p", [128, 770], F16, kind="ExternalInput").ap()
    # cols 0:16 = per (layer, m-half) ReLU bias; col 16 = bsdf
    biasm = nc.dram_tensor("biasm", [128, 17], F32, kind="ExternalInput").ap()
    out_o = nc.dram_tensor("out_o", [NPC // NT, NT], F32, kind="ExternalOutput").ap()

    with tile.TileContext(nc) as tc:
        with (
            tc.tile_pool(name="wp", bufs=1) as wp,
            tc.tile_pool(name="ep", bufs=4) as ep,
            tc.tile_pool(name="hp", bufs=5) as hp,
            tc.tile_pool(name="op", bufs=4) as op_,
            tc.tile_pool(name="pp", bufs=6, space="PSUM") as pp,
            tc.tile_pool(name="pf", bufs=1, space="PSUM") as pf,
        ):
            # ---- weight/const loads: embedding tile 0 first on the sync
            # queue; weights issue in parallel on the idle GpSimd queue ----
            embt0 = ep.tile([128, NT], F16, tag="embt")  # ptile 0 embedding
            nc.sync.dma_start(out=embt0, in_=embp[:, 0:NT])
            w0s = wp.tile_from(w0d, name="w0s")
            bs_ = wp.tile_from(biasm, name="bs_")
            wks = {}
            for i in (1, 2, 3):
                wks[i] = wp.tile_from(wkh[i], name=f"wks{i}")  # [128, 512]
            w4s = wp.tile_from(w4p, name="w4s")
            for i in (5, 6, 7):
                wks[i] = wp.tile_from(wkh[i], name=f"wks{i}")

            def wchunk(i, c, m):
                # layer i (1..7, not 4), K-chunk c, M-half m -> [128, 128]
                return wks[i][:, _ts(2 * c + m, 128)]

            w4e_a = w4s[0:64, :]     # rows 0:39 used (A set)
            w4e_b = w4s[64:128, :]   # rows 64:103 used (B set)
            wsdf_a = w4s[:, 768:769]
            wsdf_b = w4s[:, 769:770]

            def relu_q(dst, src, li, m):
                # ReLU+bias for one [128, 512] quarter; m0 -> ACT, m1 -> DVE
                bias = bs_[:, _ts(2 * li + m, 1)]
                if m == 0:
                    nc.scalar.activation(dst, src, AF.Relu, bias=bias)
                else:
                    nc.vector.tensor_scalar(
                        dst, src, bias, 0.0, op0=ALU.add, op1=ALU.max
                    )

            pending = None  # deferred SDF drain: (psfa, psfb, t)

            def flush_out(pending):
                psfa, psfb, tp = pending
                # both drains on ACT (keeps DVE free for ReLUs)
                oa = op_.tile([1, NT], F32, tag="oa", name="oa")
                nc.scalar.activation(oa, psfa, AF.Identity, bias=bs_[0:1, 16:17])
                ob_t = op_.tile([33, NT], F32, tag="ob", name="ob_t")
                nc.scalar.activation(
                    ob_t[32:33, :], psfb, AF.Identity, bias=bs_[32:33, 16:17]
                )
                nc.sync.dma_start(out=out_o[2 * tp:2 * tp + 1, :], in_=oa)
                nc.sync.dma_start(
                    out=out_o[2 * tp + 1:2 * tp + 2, :], in_=ob_t[32:33, :]
                )

            # tile state: t -> {"embt": tile, "h": {li: quarter dict}}
            st = {}

            def emit_layer(t, li):
                S = st[t]
                embt = S["embt"]
                h = {
                    (s, m): hp.tile(
                        [128, NT], F16, tag=f"h{s}{m}", name=f"h{s}{m}_{li}"
                    )
                    for s in range(2) for m in range(2)
                }
                if li == 0:
                    # concurrent row-group pairs (A rows 0-63, B 64-127)
                    for m in range(2):
                        pa = pp.tile([128, NT], F32, tag="ps", name=f"pA{m}")
                        pb = pp.tile([128, NT], F32, tag="ps", name=f"pB{m}")
                        nc.tensor.matmul(
                            pa, w0s[0:64, _ts(m, 128)],
                            embt[0:64, :], start=True, stop=True,
                        )
                        nc.tensor.matmul(
                            pb, w0s[64:128, _ts(m, 128)],
                            embt[64:128, :], start=True, stop=True,
                        )
                        relu_q(h[(0, m)], pa, li, m)
                        relu_q(h[(1, m)], pb, li, m)
                elif li == 4:
                    # E-chunk first (concurrent row-group pairs), opens
                    # each bank's accumulation group
                    h3 = S["h"][3]
                    ps4 = {}
                    for m in range(2):
                        ps4[(0, m)] = pp.tile(
                            [128, NT], F32, tag="ps", name=f"p4A{m}"
                        )
                        ps4[(1, m)] = pp.tile(
                            [128, NT], F32, tag="ps", name=f"p4B{m}"
                        )
                        nc.tensor.matmul(
                            ps4[(0, m)], w4e_a[:, _ts(m, 128)],
                            embt[0:64, :], start=True, stop=False,
                            skip_group_check=True,
                        )
                        nc.tensor.matmul(
                            ps4[(1, m)], w4e_b[:, _ts(m, 128)],
                            embt[64:128, :], start=True, stop=False,
                            skip_group_check=True,
                        )
                    for s in range(2):
                        for m in range(2):
                            for c in range(2):
                                lhsT = w4s[:, _ts(2 + 2 * c + m, 128)]
                                nc.tensor.matmul(
                                    ps4[(s, m)], lhsT, h3[(s, c)],
                                    start=False, stop=(c == 1),
                                    skip_group_check=True,
                                )
                            relu_q(h[(s, m)], ps4[(s, m)], li, m)
                else:
                    hprev = S["h"][li - 1]
                    for s in range(2):
                        for m in range(2):
                            pq = pp.tile(
                                [128, NT], F32, tag="ps", name=f"p{s}{m}"
                            )
                            for c in range(2):
                                nc.tensor.matmul(
                                    pq, wchunk(li, c, m), hprev[(s, c)],
                                    start=(c == 0), stop=(c == 1),
                                )
                            relu_q(h[(s, m)], pq, li, m)
                S["h"][li] = h

            def emit_sdf(t):
                # final SDF layer (M=1): A chain at array col 0, B chain at
                # col 32, separate PSUM banks
                h7 = st[t]["h"][7]
                psfa = pf.tile([1, NT], F32, tag="finA")
                psfb_t = pf.tile([33, NT], F32, tag="finB")
                psfb = psfb_t[32:33, :]
                nc.tensor.matmul(
                    psfa, wsdf_a, h7[(0, 0)],
                    start=True, stop=False, tile_position=(0, 0),
                    skip_group_check=True,
                )
                nc.tensor.matmul(
                    psfb, wsdf_a, h7[(1, 0)],
                    start=True, stop=False, tile_position=(0, 32),
                    skip_group_check=True,
                )
                nc.tensor.matmul(
                    psfa, wsdf_b, h7[(0, 1)],
                    start=False, stop=True, tile_position=(0, 0),
                    skip_group_check=True,
                )
                nc.tensor.matmul(
                    psfb, wsdf_b, h7[(1, 1)],
                    start=False, stop=True, tile_position=(0, 32),
                    skip_group_check=True,
                )
                del st[t]
                return (psfa, psfb, t)

            # ---- staggered 2-tile software pipeline ----
            # tile t: L0-L3 in slot t, L4-L7 in slot t+1, SDF in slot t+2.
            st[0] = {"embt": embt0, "h": {}}
            for s in range(NTILES + 2):
                if s + 1 < NTILES:  # prefetch embedding one slot ahead
                    et = ep.tile([128, NT], F16, tag="embt", name=f"embt{s + 1}")
                    nc.sync.dma_start(out=et, in_=embp[:, _ts(s + 1, NT)])
                    st[s + 1] = {"embt": et, "h": {}}
                a, b = s, s - 1  # tile a runs L0-L3, tile b runs L4-L7
                if a < NTILES:
                    emit_layer(a, 0)
                if pending is not None:
                    flush_out(pending)
                    pending = None
                for k in range(1, 4):
                    if 0 <= b < NTILES:
                        emit_layer(b, 3 + k)
                    if a < NTILES:
                        emit_layer(a, k)
                if 0 <= b < NTILES:
                    emit_layer(b, 7)
                    pending = emit_sdf(b)
            if pending is not None:
                flush_out(pending)
    nc.compile()
    return nc


def _prep_maps(points, ws, bs, wsdf, bsdf):
    pts = np.ascontiguousarray(points, dtype=np.float32).reshape(N, 3)
    freqs = (2.0 ** np.arange(NHARM)).astype(np.float64)

    w0d = np.zeros((128, 256), np.float16)
    w0d[0:E] = ws[0].astype(np.float16)
    w0d[64:64 + E] = ws[0].astype(np.float16)
    w4p = np.zeros((128, 770), np.float16)
    w4p[0:E, 0:256] = ws[4][0:E].astype(np.float16)
    w4p[64:64 + E, 0:256] = ws[4][0:E].astype(np.float16)
    # K-chunk c, M-half m at cols 256 + (2c+m)*128
    for c in range(2):
        chunk = ws[4][E + 128 * c:E + 128 * (c + 1)].astype(np.float16)
        for m in range(2):
            w4p[:, 256 + (2 * c + m) * 128:256 + (2 * c + m + 1) * 128] = \
                chunk[:, m * 128:(m + 1) * 128]
    w4p[:, 768:769] = wsdf[0:128].astype(np.float16)
    w4p[:, 769:770] = wsdf[128:256].astype(np.float16)

    wk = {}
    for i in (1, 2, 3, 5, 6, 7):
        wi = ws[i].astype(np.float16)
        packed = np.empty((128, 512), np.float16)
        for c in range(2):
            for m in range(2):
                packed[:, (2 * c + m) * 128:(2 * c + m + 1) * 128] = \
                    wi[128 * c:128 * (c + 1), m * 128:(m + 1) * 128]
        wk[i] = packed

    biasm = np.zeros((128, 17), np.float32)
    for i in range(8):
        biasm[:, 2 * i] = bs[i][0:128]
        biasm[:, 2 * i + 1] = bs[i][128:256]
    biasm[:, 16] = float(np.ravel(bsdf)[0])

    common = {"w0d": w0d, "w4p": w4p, "biasm": biasm}
    for i in (1, 2, 3, 5, 6, 7):
        common[f"wk{i}"] = wk[i]

    in_maps = []
    for cix in range(N_CORES):
        sl = pts[cix * NPC:(cix + 1) * NPC].astype(np.float64)  # [NPC, 3]
        e18 = (sl[:, :, None] * freqs).reshape(NPC, 18)
        emb39 = np.concatenate(
            [np.sin(e18), np.cos(e18), sl], axis=1
        ).astype(np.float16)  # [NPC, 39]
        Er = np.ascontiguousarray(emb39.T).reshape(E, NTILES, 2, NT)
        embp = np.zeros((128, NPC // 2), np.float16)
        embp[0:E] = Er[:, :, 0, :].reshape(E, -1)
        embp[64:64 + E] = Er[:, :, 1, :].reshape(E, -1)
        m = dict(common)
        m["embp"] = embp
        in_maps.append(m)
    return in_maps


def _prep_in_maps(inputs):
    ws = [np.asarray(inputs[f"w{i}"], dtype=np.float32) for i in range(8)]
    bs = [np.asarray(inputs[f"b{i}"], dtype=np.float32) for i in range(8)]
    return _prep_maps(
        np.asarray(inputs["points"]), ws, bs,
        np.asarray(inputs["wsdf"], dtype=np.float32),
        np.asarray(inputs["bsdf"], dtype=np.float32),
    )


def kernel(
    points, w0, b0, w1, b1, w2, b2, w3, b3, w4, b4, w5, b5, w6, b6, w7, b7,
    wsdf, bsdf,
):
    ws = [np.asarray(w, dtype=np.float32) for w in (w0, w1, w2, w3, w4, w5, w6, w7)]
    bs = [np.asarray(b, dtype=np.float32) for b in (b0, b1, b2, b3, b4, b5, b6, b7)]
    in_maps = _prep_maps(
        np.asarray(points), ws, bs,
        np.asarray(wsdf, dtype=np.float32), np.asarray(bsdf, dtype=np.float32),
    )

    if "nc" not in _CACHED:
        _CACHED["nc"] = _build()
    nc = _CACHED["nc"]

    res = run_bass_kernel_spmd(nc, in_maps, core_ids=list(range(N_CORES)))
    out = np.concatenate(
        [res.results[c]["out_o"] for c in range(N_CORES)], axis=0
    ).reshape(N, 1).astype(np.float32)
    return out
